# revision 2
# baseline (speedup 1.0000x reference)
"""BertSelfAttention (B=4, S=1024, HID=768, H=12) on 8 TRN2 NeuronCores.

Sharding: core c <- (batch b = c//2, head-block hb = c%2, heads 6*hb..6*hb+5).
Each core computes, for its 6 heads:
  scores^T = k'q'^T (+mask), softmax numerator e = exp(scores^T), qq/kk/vv
  self-score tensors, and ctx^T = v'^T e with an appended ones column that
  yields the softmax denominator as row 64.
Host prep: hidden^T per batch, per-core weight slices W.T pre-scaled by
d^-1/4 (so all score matmuls come out scaled by 1/sqrt(d)), biases, mask in
column layout. Host post: transpose scores^T, normalize context by the
denominator, add the value bias analytically (sum(probs)=1).

All matmul operands are f32r (TF32-like, 1 cycle/row vs 4 for fp32; measured
rel err ~1.5e-4). PSUM is evicted via ScalarE (projections, scores^T+mask,
exp, ctx) and VectorE (qq/kk/vv) to balance the two engines; DMA of the
~100MB/core score outputs is the roofline.
"""
import math

import numpy as np

import concourse.bacc as bacc
import concourse.mybir as mybir
import concourse.tile as tile
from concourse.bass_utils import run_bass_kernel_spmd

B, S, HID, H = 4, 1024, 768, 12
D = HID // H              # 64
H6 = 6                    # heads per core
NCORES = 8
NK = HID // 128           # 6 contract subtiles
NM = S // 128             # 8 seq chunks
SCALE = float(D) ** -0.25  # applied to Wq/Wk/Wv on host
F32 = mybir.dt.float32
F32R = mybir.dt.float32r

_CACHE = {}


def _build_nc():
    nc = bacc.Bacc()
    hT = nc.declare_dram_parameter("hT", [HID, S], F32, isOutput=False)
    w_all = nc.declare_dram_parameter("w_all", [HID, 3 * 384], F32, isOutput=False)
    b_all = nc.declare_dram_parameter("b_all", [128, 9], F32, isOutput=False)
    mask_col = nc.declare_dram_parameter("mask_col", [128, NM], F32, isOutput=False)
    sT_out = nc.declare_dram_parameter("sT_out", [H6, S, S], F32, isOutput=True)
    qq_out = nc.declare_dram_parameter("qq_out", [H6, S, S], F32, isOutput=True)
    kk_out = nc.declare_dram_parameter("kk_out", [H6, S, S], F32, isOutput=True)
    vv_out = nc.declare_dram_parameter("vv_out", [H6, S, S], F32, isOutput=True)
    ctx_out = nc.declare_dram_parameter("ctx_out", [H6, D + 1, S], F32, isOutput=True)

    Exp = mybir.ActivationFunctionType.Exp

    with tile.TileContext(nc) as tc:
        with (
            tc.tile_pool(name="const", bufs=1) as const,
            tc.tile_pool(name="stg", bufs=3) as stg,
            tc.tile_pool(name="proj", bufs=1) as proj,
            tc.tile_pool(name="stage", bufs=4) as stage,
            tc.tile_pool(name="et", bufs=3) as etp,
            tc.tile_pool(name="ps", bufs=1, space="PSUM") as ps,
        ):
            bcol = const.tile([128, 9], F32)
            nc.sync.dma_start(out=bcol[:], in_=b_all[:])
            mcol = const.tile([128, NM], F32)
            nc.sync.dma_start(out=mcol[:], in_=mask_col[:])

            # ---- load + round inputs to f32r ----
            hT_r = proj.tile([128, NK, S], F32R)
            w_r = proj.tile([128, NK, 1152], F32R)
            for kt in range(NK):
                st = stg.tile([128, 1152], F32, tag="stg")
                nc.sync.dma_start(out=st[:, :S], in_=hT[kt * 128:(kt + 1) * 128, :])
                nc.scalar.copy(hT_r[:, kt, :], st[:, :S])
                st2 = stg.tile([128, 1152], F32, tag="stg")
                nc.sync.dma_start(out=st2[:], in_=w_all[kt * 128:(kt + 1) * 128, :])
                nc.scalar.copy(w_r[:, kt, :], st2[:])

            # ---- projections: qT/kT/vT (per-head-dim rows) + v_ext (natural) ----
            qT_r = proj.tile([128, 3, S], F32R)
            kT_r = proj.tile([128, 3, S], F32R)
            vT_r = proj.tile([128, 3, S], F32R)
            v_ext = proj.tile([128, NM, H6 * (D + 1)], F32R)
            ones48 = const.tile([128, NM], F32)
            nc.gpsimd.memset(ones48[:], 1.0)
            for h in range(H6):  # ones column -> softmax denominator row
                nc.scalar.copy(v_ext[:, :, h * (D + 1) + D: (h + 1) * (D + 1)],
                               ones48[:].rearrange("p (a b) -> p a b", b=1))

            for pj, dest in ((0, qT_r), (1, kT_r), (2, vT_r)):
                for msub in range(3):
                    pst = ps.tile([128, S], F32, tag="mm")
                    for nh in range(2):
                        for kt in range(NK):
                            nc.tensor.matmul(
                                pst[:, nh * 512:(nh + 1) * 512],
                                w_r[:, kt, pj * 384 + msub * 128: pj * 384 + (msub + 1) * 128],
                                hT_r[:, kt, nh * 512:(nh + 1) * 512],
                                start=(kt == 0), stop=(kt == NK - 1),
                                skip_group_check=True,
                            )
                    nc.scalar.add(dest[:, msub, :], pst[:], bcol[:, pj * 3 + msub: pj * 3 + msub + 1])

            for m in range(NM):
                pst = ps.tile([128, 384], F32, tag="vnat")
                for kt in range(NK):
                    nc.tensor.matmul(
                        pst[:], hT_r[:, kt, m * 128:(m + 1) * 128], w_r[:, kt, 768:1152],
                        start=(kt == 0), stop=(kt == NK - 1), skip_group_check=True,
                    )
                for h in range(H6):
                    nc.scalar.copy(v_ext[:, m, h * (D + 1): h * (D + 1) + D],
                                   pst[:, h * D:(h + 1) * D])

            # ---- per-head scores + softmax + context ----
            for h in range(H6):
                p3, ro = h // 2, 64 * (h % 2)

                for t_r, out_dram in ((qT_r, qq_out), (kT_r, kk_out), (vT_r, vv_out)):
                    for m in range(NM):
                        pst = ps.tile([128, S], F32, tag="mm")
                        for nh in range(2):
                            nc.tensor.matmul(
                                pst[:, nh * 512:(nh + 1) * 512],
                                t_r[ro:ro + 64, p3, m * 128:(m + 1) * 128],
                                t_r[ro:ro + 64, p3, nh * 512:(nh + 1) * 512],
                                start=True, stop=True, skip_group_check=True,
                            )
                        sg = stage.tile([128, S], F32, tag="score")
                        nc.vector.tensor_copy(sg[:], pst[:])
                        nc.sync.dma_start(out=out_dram[h, m * 128:(m + 1) * 128, :], in_=sg[:])

                ctx_ps = ps.tile([D + 1, S], F32, tag="ctx")
                for kc in range(NM):
                    pst = ps.tile([128, S], F32, tag="mm")
                    for qh in range(2):
                        nc.tensor.matmul(
                            pst[:, qh * 512:(qh + 1) * 512],
                            kT_r[ro:ro + 64, p3, kc * 128:(kc + 1) * 128],
                            qT_r[ro:ro + 64, p3, qh * 512:(qh + 1) * 512],
                            start=True, stop=True, skip_group_check=True,
                        )
                    sg = stage.tile([128, S], F32, tag="score")
                    nc.scalar.add(sg[:], pst[:], mcol[:, kc:kc + 1])  # scores^T + mask
                    nc.sync.dma_start(out=sT_out[h, kc * 128:(kc + 1) * 128, :], in_=sg[:])
                    e_t = etp.tile([128, S], F32R, tag="et")
                    nc.scalar.activation(e_t[:], sg[:], Exp)
                    for qh in range(2):
                        nc.tensor.matmul(
                            ctx_ps[:, qh * 512:(qh + 1) * 512],
                            v_ext[:, kc, h * (D + 1):(h + 1) * (D + 1)],
                            e_t[:, qh * 512:(qh + 1) * 512],
                            start=(kc == 0), stop=(kc == NM - 1), skip_group_check=True,
                        )
                cg = stage.tile([D + 1, S], F32, tag="ctxs")
                nc.scalar.copy(cg[:], ctx_ps[:])
                nc.sync.dma_start(out=ctx_out[h], in_=cg[:])

    nc.compile()
    return nc


def _get_nc():
    if "nc" not in _CACHE:
        _CACHE["nc"] = _build_nc()
    return _CACHE["nc"]


def kernel(hidden_states, attention_mask, Wq, bq, Wk, bk, Wv, bv):
    hidden_states = np.ascontiguousarray(np.asarray(hidden_states, dtype=np.float32))
    attention_mask = np.asarray(attention_mask, dtype=np.float32)
    Wq = np.asarray(Wq, dtype=np.float32)
    Wk = np.asarray(Wk, dtype=np.float32)
    Wv = np.asarray(Wv, dtype=np.float32)
    bq = np.asarray(bq, dtype=np.float32)
    bk = np.asarray(bk, dtype=np.float32)
    bv = np.asarray(bv, dtype=np.float32)

    nc = _get_nc()

    in_maps = []
    for c in range(NCORES):
        b, hb = c // 2, c % 2
        rs = slice(384 * hb, 384 * hb + 384)
        hT = np.ascontiguousarray(hidden_states[b].T)
        w_all = np.ascontiguousarray(
            np.concatenate([Wq[rs].T, Wk[rs].T, Wv[rs].T], axis=1) * SCALE)
        b_cat = np.concatenate([bq[rs], bk[rs], bv[rs]]) * SCALE
        b_all = np.ascontiguousarray(b_cat.reshape(9, 128).T)
        mask = np.ascontiguousarray(
            attention_mask[b, 0, 0].reshape(NM, 128).T)
        in_maps.append({"hT": hT, "w_all": w_all, "b_all": b_all, "mask_col": mask})

    res = run_bass_kernel_spmd(nc, in_maps, list(range(NCORES)))

    context = np.empty((B, S, HID), dtype=np.float32)
    att = np.empty((B, H, S, S), dtype=np.float32)
    qq = np.empty((B, H, S, S), dtype=np.float32)
    kk = np.empty((B, H, S, S), dtype=np.float32)
    vv = np.empty((B, H, S, S), dtype=np.float32)
    d4 = float(D) ** 0.25

    for c in range(NCORES):
        b, hb = c // 2, c % 2
        r = res.results[c]
        hs = slice(6 * hb, 6 * hb + 6)
        att[b, hs] = r["sT_out"].transpose(0, 2, 1)
        qq[b, hs] = r["qq_out"]
        kk[b, hs] = r["kk_out"]
        vv[b, hs] = r["vv_out"]
        co = r["ctx_out"]  # [6, 65, 1024]
        for h in range(H6):
            hg = 6 * hb + h
            sigma = co[h, D]                      # [S] softmax denominator
            ctxT = co[h, :D]                      # [D, S]
            context[b, :, hg * D:(hg + 1) * D] = (
                ctxT.T * (d4 / sigma)[:, None] + bv[hg * D:(hg + 1) * D][None, :])

    context_score = np.zeros((), dtype=np.float32)
    return (context, att, vv, context_score, qq, kk)
